# revision 23
# baseline (speedup 1.0000x reference)
"""GATv2Conv(64, 1024, heads=16) + Linear(16384, 20) Trainium2 kernel.

Shard by destination node, 512 nodes/core on 8 cores, with host-side
node-to-tile balancing so every node-tile has exactly 128 nodes and 640
incoming edges (zero padding; EPC=2560 edges/core).

Logit path, per 512-edge chunk (channel-major):
  The logit is sum_c a_c lrelu(v_c), v = xl[src]+xr[dst].  lrelu(v) =
  0.6v + 0.4|v|; the linear part is host-baked per edge (rank-16 node
  factors sl/sr).  The |v| part is split per head by |a_c|*sigma_c:
  - exact top 62.5% (5 blocks of 128 per head, 40 pairs): per pair j
      Y2 = W2cb^T @ xcatT  (PE bf16, out [128ch, 512e] PSUM)
      r8 = |Y2| (Act) or relu(Y2) (DVE) -> fp8
      psT += s8_j^T @ r8   (PE fp8 DoubleRow, out [16h, 512e])
  - quad bottom 37.5%: |v| ~ 0.3989(sigma + v^2/sigma), so
      0.4 sum_Q a_c|v_c| ~ qconst_h + f^T M_h f  with
      M_h = 0.1596 sum_Q (a_c/sigma_c) w_c w_c^T  (host eigendecomp).
      On chip: p = wq_h^T @ xcatT (PE), q8 = p^2 -> fp8 (Act/DVE),
      psT += sq8^T @ q8 (fp8 DoubleRow, weights exactly +-1 because the
      host folds sqrt(320|lambda|) into wq and 320 = 1/LSCALE).
      qconst is baked into baseT (per edge-type: self-loop vs regular).
  Then L = psT + baseT (DVE), PE-transpose 128-edge blocks, exp (Act)
  -> P [128e, 16h]. Segment softmax via 0/1 matrices on PE as before.

Aggregation: G (alpha-weighted x sums) with 2-heads-per-matmul packing,
agg = Wl^T G per channel chunk for all 4 node-tiles in one matmul
(4-tile-packed rhs), relu+bias (Act/DVE), z accumulated on PE.
"""

import numpy as np

N_NODES = 4096
N_EDGES = 16384
F_IN = 64
H = 16
C = 1024
HC = H * C
N_CLASS = 20
N_CORES = 8
TPC = 4  # node-tiles per core
NT = 128  # nodes per tile
ET = 640  # edges per tile (exact, balanced)
SUBT = 5  # subtiles (128 edges) per tile
NSUB = TPC * SUBT  # 20
EPC = TPC * ET  # 2560 edges per core
ECH = 512  # edges per chunk
NCHK = EPC // ECH  # 5

B_X = 5  # exact blocks (of 128 ch) per head
NBLK = H * B_X  # 80 exact blocks
NPAIR_X = NBLK // 2  # 40 exact channel-block pairs
KQ = 1280  # kept quad eigencolumns (global top-|lambda|); tail mean-baked
NQ = KQ // 256  # 5 quad tiles (256 eigencolumns each)
CX = B_X * 128  # exact channels per head (640)

NCK = HC // 128  # 128 channel chunks for agg/z
FP8_S = 256.0  # W2 pre-scale so relu(Y2) fits fp8 e4m3
LSCALE = 0.8 / FP8_S  # logit scale applied in the exp
SQ_SCALE = 1.0 / LSCALE  # 320: folded into wq so sq8 weights are +-1
QA = 0.3989422804014327  # |t| ~ QA + QB t^2 fit for t ~ N(0,1)
QB = 0.3989422804014327

# Per-chunk op schedule: 40 exact pairs + 5 quad tiles, with engine
# assignment (Act abs / DVE relu for pairs; squares always Act — DVE
# cannot self-mult per the BIR verifier).  Act gets 20 pairs + 5
# squares, DVE 20 pairs, interleaved so neither engine sees long
# same-engine runs.  Entries: ('x', j, 'A'|'D') / ('q', qi, 'A').
N_PA = 20  # Act pairs
def _build_schedule():
    sched = []
    xj = 0
    qi = 0
    # 45 slots; quad tiles (Act) every 8 slots from 7 (late enough that
    # the wq DMA has landed before chunk-0 needs it)
    quad_slots = {7 + 8 * i for i in range(NQ)}
    na = nd = 0
    n_pd = NPAIR_X - N_PA
    for slot in range(NPAIR_X + NQ):
        if (slot in quad_slots and qi < NQ) or xj >= NPAIR_X:
            sched.append(('q', qi, 'A'))
            qi += 1
            continue
        pick = 'A' if na * n_pd <= nd * N_PA else 'D'
        if pick == 'A' and na >= N_PA:
            pick = 'D'
        if pick == 'D' and nd >= n_pd:
            pick = 'A'
        if pick == 'A':
            na += 1
        else:
            nd += 1
        sched.append(('x', xj, pick))
        xj += 1
    return sched


SCHED = _build_schedule()
# engine per exact pair (for host-side w/coef baking)
PAIR_ENG = {}
for kind, idx, eng in SCHED:
    if kind == 'x':
        PAIR_ENG[idx] = eng

_CACHE = {}


def _build_nc():
    import concourse.bacc as bacc
    import concourse.bass as bass
    import concourse.mybir as mybir
    import concourse.tile as tile

    f32 = mybir.dt.float32
    bf16 = mybir.dt.bfloat16
    fp8 = mybir.dt.float8e4
    AF = mybir.ActivationFunctionType
    OP = mybir.AluOpType
    PM = mybir.MatmulPerfMode

    nc = bacc.Bacc("TRN2", target_bir_lowering=False)

    W2W = NPAIR_X * 256  # 10240 exact-channel columns
    d_xT = nc.dram_tensor("xT", [128, EPC], bf16, kind="ExternalInput")
    d_w2 = nc.dram_tensor("w2", [128, W2W], bf16, kind="ExternalInput")
    d_wq = nc.dram_tensor("wq", [128, NQ, 2, 128], bf16, kind="ExternalInput")
    d_s8 = nc.dram_tensor("s8", [128, NPAIR_X, 2, H], fp8, kind="ExternalInput")
    d_sq8 = nc.dram_tensor("sq8", [128, NQ, 2, H], fp8, kind="ExternalInput")
    d_baseT = nc.dram_tensor("baseT", [16, EPC], f32, kind="ExternalInput")
    d_ident = nc.dram_tensor("ident", [16, 16], f32, kind="ExternalInput")
    d_s01t = nc.dram_tensor("s01t", [128, NSUB, NT], bf16, kind="ExternalInput")
    d_s01n = nc.dram_tensor("s01n", [128, NSUB, 128], bf16, kind="ExternalInput")
    d_xsrc = nc.dram_tensor("xsrc", [128, NSUB, F_IN], bf16, kind="ExternalInput")
    d_wagg = nc.dram_tensor("wagg", [128, 64, 128], bf16, kind="ExternalInput")
    d_wout = nc.dram_tensor("wout", [128, NCK, N_CLASS], bf16, kind="ExternalInput")
    d_biasc = nc.dram_tensor("biasc", [128, NCK], f32, kind="ExternalInput")
    d_z = nc.dram_tensor("z", [128, TPC, N_CLASS], f32, kind="ExternalOutput")

    with tile.TileContext(nc) as tc:
        with (
            tc.tile_pool(name="const", bufs=1) as cpool,
            tc.tile_pool(name="r8", bufs=8) as r8pool,
            tc.tile_pool(name="lbuf", bufs=3) as lpool,
            tc.tile_pool(name="xw", bufs=40) as xwpool,
            tc.tile_pool(name="rt", bufs=10) as rtpool,
            tc.tile_pool(name="rec", bufs=2) as recpool,
            tc.tile_pool(name="aux", bufs=1, space=bass.MemorySpace.PSUM) as auxp,
            tc.tile_pool(name="psT", bufs=1, space=bass.MemorySpace.PSUM) as psTp,
        ):
            xT = cpool.tile([128, EPC], bf16)
            w2 = cpool.tile([128, W2W], bf16)
            wq = cpool.tile([128, NQ, 2, 128], bf16)
            s8 = cpool.tile([128, NPAIR_X, 2, H], fp8)
            sq8 = cpool.tile([128, NQ, 2, H], fp8)
            baseT = cpool.tile([16, EPC], f32)
            ident = cpool.tile([16, 16], f32)
            s01t = cpool.tile([128, NSUB, NT], bf16)
            s01n = cpool.tile([128, NSUB, 128], bf16)
            xsrc = cpool.tile([128, NSUB, F_IN], bf16)
            wagg = cpool.tile([128, 64, 128], bf16)
            wout = cpool.tile([128, NCK, N_CLASS], bf16)
            biasc = cpool.tile([128, NCK], f32)
            # first-needed inputs first; w2 split so chunk-0 mains start
            # before the whole array arrives.  ident goes first (tiny) so
            # the act-table warmup below runs before the big DMAs land.
            nc.sync.dma_start(ident[:], d_ident[:])
            nc.sync.dma_start(xT[:, 0:ECH], d_xT[:, 0:ECH])
            nc.sync.dma_start(w2[:, 0:512], d_w2[:, 0:512])
            nc.sync.dma_start(s8[:], d_s8[:])
            nc.sync.dma_start(sq8[:], d_sq8[:])
            nc.sync.dma_start(wq[:], d_wq[:])
            nc.sync.dma_start(w2[:, 512:2048], d_w2[:, 512:2048])
            for kq in range(1, 5):
                nc.sync.dma_start(
                    w2[:, kq * 2048 : (kq + 1) * 2048],
                    d_w2[:, kq * 2048 : (kq + 1) * 2048],
                )
            nc.sync.dma_start(xT[:, ECH:], d_xT[:, ECH:])
            for sb, dr in [
                (baseT, d_baseT), (s01t, d_s01t),
                (s01n, d_s01n), (xsrc, d_xsrc), (wagg, d_wagg),
                (wout, d_wout), (biasc, d_biasc),
            ]:
                nc.sync.dma_start(sb[:], dr[:])

            # act-table warmup: a dummy Abs on ident pulls the (single)
            # LoadActFuncSet off the chunk-0 critical path; dummy matmuls
            # start the PE p-state ramp clock while the first DMAs land
            warm = cpool.tile([16, 16], f32)
            nc.scalar.activation(warm[:], ident[:], AF.Abs)

            P_sb = cpool.tile([128, NSUB, H], bf16)
            alpha_sb = cpool.tile([128, TPC, SUBT * H], f32)
            ssb = cpool.tile([128, TPC, H], bf16)
            G_sb = cpool.tile([128, TPC, 8, NT], bf16)
            z_sb = cpool.tile([128, TPC, N_CLASS], f32)

            def tile_tail(t):
                """den, reciprocal, alpha, xw, G, G-copy for node-tile t.

                For the last tile (t==3) the logit phase is over, so Act
                and DVE are idle: split the xw gathers three ways to get
                G_sb ready (and the agg phase started) as fast as possible.
                """
                last = t == TPC - 1
                with tc.high_priority():
                    sst_t = auxp.tile([128, 512], f32, tag="aux")
                    sst = sst_t[:, 0:H]
                    for s2 in range(SUBT):
                        nc.tensor.matmul(
                            sst,
                            s01t[:, t * SUBT + s2, :],
                            P_sb[:, t * SUBT + s2, :],
                            start=(s2 == 0),
                            stop=(s2 == SUBT - 1),
                        )
                    nc.vector.tensor_copy(ssb[:, t, :], sst)
                    # den: 5 single-shot matmuls, one-start in one aux alloc
                    dsp_t = auxp.tile([128, 512], f32, tag="aux")
                    dsp = dsp_t[:, 0 : SUBT * H].rearrange("p (a b) -> p a b", a=SUBT)
                    for s2 in range(SUBT):
                        nc.tensor.matmul(
                            dsp[:, s2, :],
                            s01n[:, t * SUBT + s2, :],
                            ssb[:, t, :],
                            start=(s2 == 0), stop=(s2 == SUBT - 1),
                        )
                    rec = recpool.tile([128, SUBT * H], f32, tag="rec")
                    nc.vector.reciprocal(rec[:], dsp_t[:, 0 : SUBT * H])
                    # alpha on Pool (SBUF-only engine work)
                    nc.gpsimd.tensor_tensor(
                        out=alpha_sb[:, t, :],
                        in0=P_sb[:, t * SUBT : (t + 1) * SUBT, :],
                        in1=rec[:],
                        op=OP.mult,
                    )
                # all alpha-scaled source gathers on Pool first, then the
                # G matmuls, so the span-ring slot is held briefly
                xws = {}
                nxw = [0]
                for pair in range(8):
                    for s2 in range(SUBT):
                        xw = xwpool.tile([128, 2, F_IN], bf16, tag="xw")
                        xws[pair, s2] = xw
                        for q2 in range(2):
                            h = 2 * pair + q2
                            al = alpha_sb[:, t, s2 * H + h : s2 * H + h + 1]
                            i = nxw[0]
                            nxw[0] += 1
                            # last tile: Act/DVE are idle, take some gathers
                            # (53/238/192 ns per op on Pool/Act/DVE)
                            if not last or i % 4 < 2:
                                nc.gpsimd.tensor_scalar_mul(
                                    xw[:, q2, :], xsrc[:, t * SUBT + s2, :], al
                                )
                            elif i % 4 == 2:
                                nc.scalar.mul(
                                    xw[:, q2, :], xsrc[:, t * SUBT + s2, :], al
                                )
                            else:
                                nc.vector.tensor_scalar(
                                    out=xw[:, q2, :],
                                    in0=xsrc[:, t * SUBT + s2, :],
                                    scalar1=al, scalar2=None, op0=OP.mult,
                                )
                # G in two single-bank halves (pairs 0-3, 4-7); one
                # start/stop per half, copied out as soon as each is done
                for gh in range(2):
                    G_t = auxp.tile([128, 512], f32, tag="aux")
                    G = G_t[:].rearrange("p (a b) -> p a b", a=4)
                    for ph in range(4):
                        pair = 4 * gh + ph
                        for s2 in range(SUBT):
                            nc.tensor.matmul(
                                G[:, ph, :],
                                xws[pair, s2][:].rearrange("p a b -> p (a b)"),
                                s01t[:, t * SUBT + s2, :],
                                start=(ph == 0 and s2 == 0),
                                stop=(ph == 3 and s2 == SUBT - 1),
                            )
                    if (2 * t + gh) % 2 == 0:
                        nc.scalar.copy(G_sb[:, t, 4 * gh : 4 * gh + 4, :], G[:])
                    else:
                        nc.vector.tensor_copy(G_sb[:, t, 4 * gh : 4 * gh + 4, :], G[:])

            # ---------------- logit phase, per 512-edge chunk ----------------
            PIPE = 3  # psT reduce trails the span/elementwise by 3 slots
            logit_pool = tc.tile_pool(name="span", bufs=3, space=bass.MemorySpace.PSUM)
            spanp = logit_pool.__enter__()
            for ck in range(NCHK):
                psT_t = psTp.tile([16, ECH], f32, tag="psT")
                psT = psT_t[:]
                e0 = ck * ECH
                nsc = len(SCHED)
                r8s = {}
                for si in range(nsc + PIPE):
                    if si < nsc:
                        kind, idx, eng = SCHED[si]
                        span = spanp.tile([128, 2, ECH], f32, tag="span")
                        r8 = r8pool.tile([128, 2, ECH], fp8, tag="r8")
                        if kind == 'x':
                            for kt in range(2):
                                cb = 2 * idx + kt
                                nc.tensor.matmul(
                                    span[:, kt, :],
                                    w2[:, cb * 128 : (cb + 1) * 128],
                                    xT[:, e0 : e0 + ECH],
                                )
                            if eng == 'A':
                                nc.scalar.activation(
                                    r8[:].rearrange("p a b -> p (a b)"),
                                    span[:].rearrange("p a b -> p (a b)"),
                                    AF.Abs,
                                )
                            else:
                                nc.vector.tensor_scalar(
                                    out=r8[:].rearrange("p a b -> p (a b)"),
                                    in0=span[:].rearrange("p a b -> p (a b)"),
                                    scalar1=0.0, scalar2=None, op0=OP.max,
                                )
                            r8s[si] = (r8, s8[:, idx, :, :])
                        else:
                            for kt in range(2):
                                nc.tensor.matmul(
                                    span[:, kt, :],
                                    wq[:, idx, kt, :],
                                    xT[:, e0 : e0 + ECH],
                                )
                            nc.scalar.activation(
                                r8[:].rearrange("p a b -> p (a b)"),
                                span[:].rearrange("p a b -> p (a b)"),
                                AF.Square,
                            )
                            r8s[si] = (r8, sq8[:, idx, :, :])
                    if si >= PIPE:
                        r8p, red = r8s.pop(si - PIPE)
                        nc.tensor.matmul(
                            psT,
                            red,
                            r8p[:],
                            start=(si - PIPE == 0),
                            stop=(si - PIPE == nsc - 1),
                            perf_mode=PM.DoubleRow,
                        )
                # chunk drain: logits -> P, seg-sums; tile tail when complete
                with tc.high_priority():
                    Lb = lpool.tile([16, ECH], f32, tag="lbuf")
                    nc.vector.tensor_tensor(
                        out=Lb[:], in0=psT,
                        in1=baseT[:, e0 : e0 + ECH], op=OP.add,
                    )
                    ptrt_t = psTp.tile([128, 512], f32, tag="psT")
                    ptrt = ptrt_t[:, 0:64].rearrange("p (a b) -> p a b", a=4)
                    for i in range(4):
                        nc.tensor.matmul(
                            ptrt[:, i, :],
                            Lb[:, i * 128 : (i + 1) * 128], ident[:],
                            is_transpose=True,
                            start=(i == 0), stop=(i == 3),
                        )
                    nc.scalar.activation(
                        P_sb[:, 4 * ck : 4 * ck + 4, :],
                        ptrt[:],
                        AF.Exp,
                        scale=LSCALE,
                    )
                # tails run at normal priority so G matmuls don't preempt
                # the span pipeline on the PE
                for i in range(4):
                    s = 4 * ck + i
                    t, sid = divmod(s, SUBT)
                    if sid == SUBT - 1:
                        tile_tail(t)

            # ---------------- aggregation phase ----------------
            logit_pool.__exit__(None, None, None)
            nz = [0]
            zp_t = auxp.tile([128, 512], f32, tag="aux")
            zp = zp_t[:, 0 : TPC * N_CLASS]
            rts = {}
            ZLAG = 2
            with tc.tile_pool(
                name="aggsp", bufs=6, space=bass.MemorySpace.PSUM
            ) as aggsp:
                for kk in range(NCK + ZLAG):
                    if kk < NCK:
                        span = aggsp.tile([128, TPC, NT], f32, tag="aspan")
                        rt = rtpool.tile([128, TPC, NT], bf16, tag="rt")
                        rts[kk] = rt
                        q2 = (kk // 8) % 2
                        jcol = (kk // 16) * 8 + kk % 8
                        pair = (kk // 8) // 2
                        nc.tensor.matmul(
                            span[:],
                            wagg[q2 * 64 : (q2 + 1) * 64, jcol, :],
                            G_sb[q2 * 64 : (q2 + 1) * 64, :, pair, :],
                        )
                        if (kk % 15) % 2 == 0:
                            nc.scalar.activation(
                                rt[:].rearrange("p a b -> p (a b)"),
                                span[:].rearrange("p a b -> p (a b)"),
                                AF.Relu,
                                bias=biasc[:, kk : kk + 1],
                            )
                        else:
                            nc.vector.tensor_scalar(
                                out=rt[:].rearrange("p a b -> p (a b)"),
                                in0=span[:].rearrange("p a b -> p (a b)"),
                                scalar1=biasc[:, kk : kk + 1],
                                scalar2=0.0, op0=OP.add, op1=OP.max,
                            )
                    # z matmuls trail so PE never waits on the relus
                    if kk >= ZLAG:
                        kz = kk - ZLAG
                        rtp = rts.pop(kz)
                        for t in range(TPC):
                            nz[0] += 1
                            nc.tensor.matmul(
                                zp[:, t * N_CLASS : (t + 1) * N_CLASS],
                                rtp[:, t, :],
                                wout[:, kz, :],
                                start=(nz[0] == 1),
                                stop=(nz[0] == NCK * TPC),
                            )
            nc.vector.tensor_copy(
                z_sb[:].rearrange("p a b -> p (a b)"),
                zp[:, 0 : TPC * N_CLASS],
            )
            nc.sync.dma_start(d_z[:], z_sb[:])

    nc.compile()
    return nc


def _balance_tiles(deg):
    """Assign nodes to 32 tiles: 128 nodes and exactly 640 edges each."""
    T = N_CORES * TPC
    order = np.argsort(-deg, kind="stable")
    tiles = [[] for _ in range(T)]
    ebud = np.full(T, ET)
    nbud = np.full(T, NT)
    for n in order:
        cand = np.where(nbud > 0)[0]
        best = cand[np.argmax(ebud[cand] - nbud[cand])]
        tiles[best].append(int(n))
        ebud[best] -= deg[n]
        nbud[best] -= 1
    if not np.all(ebud == 0):
        rng = np.random.default_rng(0)
        for _ in range(500000):
            if np.all(ebud == 0):
                break
            hi = int(np.argmin(ebud))
            lo = int(np.argmax(ebud))
            a, b = tiles[hi], tiles[lo]
            ia = int(rng.integers(len(a)))
            ib = int(rng.integers(len(b)))
            delta = deg[a[ia]] - deg[b[ib]]
            if 0 < delta <= ebud[lo] - ebud[hi]:
                a[ia], b[ib] = b[ib], a[ia]
                ebud[hi] += delta
                ebud[lo] -= delta
    assert np.all(ebud == 0), f"tile balancing failed: {ebud}"
    return tiles


def _prep_inputs(x, edge_index, W_l, W_r, att, bias_gat, W_out, b_out):
    import ml_dtypes

    bf16 = ml_dtypes.bfloat16
    fp8 = ml_dtypes.float8_e4m3
    x = np.asarray(x, np.float32)
    W_l = np.asarray(W_l, np.float32)
    W_r = np.asarray(W_r, np.float32)
    att = np.asarray(att, np.float32)
    bias_gat = np.asarray(bias_gat, np.float32)
    W_out = np.asarray(W_out, np.float32)

    src = np.concatenate([np.asarray(edge_index[0]), np.arange(N_NODES)]).astype(
        np.int64
    )
    dst = np.concatenate([np.asarray(edge_index[1]), np.arange(N_NODES)]).astype(
        np.int64
    )
    deg = np.bincount(dst, minlength=N_NODES)
    tiles = _balance_tiles(deg)

    attf = att.reshape(HC).astype(np.float64)
    Wcat = np.vstack([W_l, W_r]).astype(np.float64)  # [128, HC]
    sig_edge = np.linalg.norm(Wcat, axis=0)  # per-channel sigma, regular edge
    Wsum = (W_l + W_r).astype(np.float64)
    sig_self = np.linalg.norm(Wsum, axis=0)  # self-loop sigma

    # ---- split channels per head: exact top CX by |a|*sigma, quad rest
    score = (np.abs(attf) * sig_edge).reshape(H, C)
    exact_idx = np.empty((H, CX), np.int64)  # global channel ids
    quad_sets = []
    for h in range(H):
        o = np.argsort(-score[h], kind="stable")
        exact_idx[h] = h * C + o[:CX]
        quad_sets.append(h * C + o[CX:])

    # exact blocks: head-major, 5 blocks per head
    blocks = np.empty((NBLK, 128), np.int64)
    blk2head = np.empty(NBLK, np.int64)
    for h in range(H):
        for j in range(B_X):
            g = h * B_X + j
            blocks[g] = exact_idx[h, j * 128 : (j + 1) * 128]
            blk2head[g] = h

    # W2: exact-channel columns, |a| and FP8_S folded
    W2 = np.empty((128, NBLK * 128), np.float64)
    for g in range(NBLK):
        ch = blocks[g]
        W2[:, g * 128 : (g + 1) * 128] = (
            Wcat[:, ch] * np.abs(attf[ch])[None, :] * FP8_S
        )
    W2 = W2.astype(bf16)

    # s8 reduce matrix: sign(a_c), scaled 0.5 for Act (abs-stored) pairs
    s8 = np.zeros((128, NPAIR_X, 2, H), np.float32)
    for j in range(NPAIR_X):
        w = 0.5 if PAIR_ENG[j] == 'A' else 1.0
        for kt in range(2):
            g = 2 * j + kt
            h = blk2head[g]
            s8[:, j, kt, h] = w * np.sign(attf[blocks[g]])
    s8 = s8.astype(fp8)

    # ---- quad path: M_h eigendecomp, keep global top-KQ |lambda| columns
    # (sqrt(320|lam|) folded into wq; sq8 weights exactly +-1), mean-bake
    # the dropped tail per edge-type into qconst.
    qconst_edge = np.zeros(H, np.float64)
    qconst_self = np.zeros(H, np.float64)
    eigs = []  # (|lam|, sign, h, u[128])
    for h in range(H):
        qc = quad_sets[h]
        aq = attf[qc]
        sq = sig_edge[qc]
        M = (0.4 * QB) * ((Wcat[:, qc] * (aq / sq)[None, :]) @ Wcat[:, qc].T)
        lam, U = np.linalg.eigh(M)
        for k in range(128):
            eigs.append((abs(lam[k]), np.sign(lam[k]), h, U[:, k]))
        qconst_edge[h] = 0.4 * QA * np.sum(aq * sq)
        qconst_self[h] = 0.4 * QA * np.sum(aq * sig_self[qc])
    eigs.sort(key=lambda e: -e[0])
    wq = np.zeros((128, NQ, 2, 128), np.float64)
    sq8 = np.zeros((128, NQ, 2, H), np.float32)
    for i, (al, sgn, h, u) in enumerate(eigs[:KQ]):
        ti, rest = divmod(i, 256)
        kt, p = divmod(rest, 128)
        wq[:, ti, kt, p] = u * np.sqrt(SQ_SCALE * al)
        sq8[p, ti, kt, h] = sgn
    for al, sgn, h, u in eigs[KQ:]:
        # E[(u^T f)^2] = 1 for independent src/dst; ||u_a+u_b||^2 for
        # self-loops (f = [x; x])
        qconst_edge[h] += sgn * al
        qconst_self[h] += sgn * al * float(np.sum((u[:64] + u[64:]) ** 2))
    wq = wq.astype(bf16)
    sq8 = sq8.astype(fp8)

    # ---- linear-part bake: coef per channel
    # 0.6 for quad channels and Act(abs)-pair channels, 0.2 for DVE(relu)
    coef = np.full(HC, 0.6, np.float64)
    for j in range(NPAIR_X):
        if PAIR_ENG[j] != 'A':
            for kt in range(2):
                coef[blocks[2 * j + kt]] = 0.2
    attc = (attf * coef).reshape(H, C)
    ul = np.einsum("fhc,hc->fh", W_l.reshape(F_IN, H, C).astype(np.float64), attc)
    ur = np.einsum("fhc,hc->fh", W_r.reshape(F_IN, H, C).astype(np.float64), attc)
    sl = x.astype(np.float64) @ ul
    sr = x.astype(np.float64) @ ur

    # wagg: chunk kk of head h=kk//8 stored at partitions (h%2)*64..+64,
    # column j=(kk//16)*8+kk%8
    wagg = np.zeros((128, 64, 128), np.float32)
    for kk in range(NCK):
        q2 = (kk // 8) % 2
        jcol = (kk // 16) * 8 + kk % 8
        wagg[q2 * 64 : (q2 + 1) * 64, jcol, :] = W_l[:, kk * 128 : (kk + 1) * 128]
    wagg = wagg.astype(bf16)

    wout = np.ascontiguousarray(
        W_out.reshape(NCK, 128, N_CLASS).transpose(1, 0, 2)
    ).astype(bf16)
    biasc = np.ascontiguousarray(bias_gat.reshape(NCK, 128).T).astype(np.float32)
    ident = np.eye(16, dtype=np.float32)

    in_maps = []
    node_map = np.empty((N_CORES, TPC, NT), np.int64)
    for core in range(N_CORES):
        xT = np.zeros((128, EPC), np.float32)
        baseT = np.zeros((16, EPC), np.float32)
        s01t = np.zeros((128, NSUB, NT), np.float32)
        s01n = np.zeros((128, NSUB, 128), np.float32)
        xsrc_a = np.zeros((128, NSUB, F_IN), np.float32)
        for tl in range(TPC):
            tg = core * TPC + tl
            nodes = np.asarray(tiles[tg], np.int64)
            node_map[core, tl] = nodes
            # local dst index per node
            loc = np.full(N_NODES, -1, np.int64)
            loc[nodes] = np.arange(NT)
            mask = loc[dst] >= 0
            mask &= np.isin(dst, nodes)
            es = src[mask]
            ed = loc[dst[mask]]
            o = np.argsort(ed, kind="stable")
            es, ed = es[o], ed[o]
            assert len(es) == ET, f"tile {tg}: {len(es)} edges"
            is_self = es == nodes[ed]
            qc = np.where(is_self[None, :], qconst_self[:, None],
                          qconst_edge[:, None])  # [H, ET]
            slot = np.arange(ET)
            ssub, p = slot // 128 + tl * SUBT, slot % 128
            e0 = tl * ET
            xT[0:64, e0 + slot] = x[es].T
            xT[64:128, e0 + slot] = x[nodes[ed]].T
            baseT[:, e0 + slot] = SQ_SCALE * (
                (sl[es] + sr[nodes[ed]]).T + qc
            )
            s01t[p, ssub, ed] = 1.0
            s01n[ed, ssub, p] = 1.0
            xsrc_a[p, ssub, :] = x[es]
        in_maps.append(
            {
                "xT": xT.astype(bf16),
                "w2": W2,
                "wq": wq,
                "s8": s8,
                "sq8": sq8,
                "baseT": baseT,
                "ident": ident,
                "s01t": s01t.astype(bf16),
                "s01n": s01n.astype(bf16),
                "xsrc": xsrc_a.astype(bf16),
                "wagg": wagg,
                "wout": wout,
                "biasc": biasc,
            }
        )
    return in_maps, node_map


def kernel(**inputs):
    from concourse.bass_utils import run_bass_kernel_spmd

    if "nc" not in _CACHE:
        _CACHE["nc"] = _build_nc()
    nc = _CACHE["nc"]

    in_maps, node_map = _prep_inputs(**inputs)
    res = run_bass_kernel_spmd(nc, in_maps, list(range(N_CORES)))
    b_out = np.asarray(inputs["b_out"], np.float32)
    z = np.empty((N_NODES, N_CLASS), np.float32)
    for core in range(N_CORES):
        zc = np.asarray(res.results[core]["z"], np.float32)  # [128, TPC, 20]
        for tl in range(TPC):
            z[node_map[core, tl]] = zc[:, tl, :]
    return z + b_out


# revision 24
# speedup vs baseline: 1.0135x; 1.0135x over previous
"""GATv2Conv(64, 1024, heads=16) + Linear(16384, 20) Trainium2 kernel.

Shard by destination node, 512 nodes/core on 8 cores, with host-side
node-to-tile balancing so every node-tile has exactly 128 nodes and 640
incoming edges (zero padding; EPC=2560 edges/core).

Logit path, per 512-edge chunk (channel-major):
  The logit is sum_c a_c lrelu(v_c), v = xl[src]+xr[dst].  lrelu(v) =
  0.6v + 0.4|v|; the linear part is host-baked per edge (rank-16 node
  factors sl/sr).  The |v| part is split per head by |a_c|*sigma_c:
  - exact top 62.5% (5 blocks of 128 per head, 40 pairs): per pair j
      Y2 = W2cb^T @ xcatT  (PE bf16, out [128ch, 512e] PSUM)
      r8 = |Y2| (Act) or relu(Y2) (DVE) -> fp8
      psT += s8_j^T @ r8   (PE fp8 DoubleRow, out [16h, 512e])
  - quad bottom 37.5%: |v| ~ 0.3989(sigma + v^2/sigma), so
      0.4 sum_Q a_c|v_c| ~ qconst_h + f^T M_h f  with
      M_h = 0.1596 sum_Q (a_c/sigma_c) w_c w_c^T  (host eigendecomp).
      On chip: p = wq_h^T @ xcatT (PE), q8 = p^2 -> fp8 (Act/DVE),
      psT += sq8^T @ q8 (fp8 DoubleRow, weights exactly +-1 because the
      host folds sqrt(320|lambda|) into wq and 320 = 1/LSCALE).
      qconst is baked into baseT (per edge-type: self-loop vs regular).
  Then L = psT + baseT (DVE), PE-transpose 128-edge blocks, exp (Act)
  -> P [128e, 16h]. Segment softmax via 0/1 matrices on PE as before.

Aggregation: G (alpha-weighted x sums) with 2-heads-per-matmul packing,
agg = Wl^T G per channel chunk for all 4 node-tiles in one matmul
(4-tile-packed rhs), relu+bias (Act/DVE), z accumulated on PE.
"""

import numpy as np

N_NODES = 4096
N_EDGES = 16384
F_IN = 64
H = 16
C = 1024
HC = H * C
N_CLASS = 20
N_CORES = 8
TPC = 4  # node-tiles per core
NT = 128  # nodes per tile
ET = 640  # edges per tile (exact, balanced)
SUBT = 5  # subtiles (128 edges) per tile
NSUB = TPC * SUBT  # 20
EPC = TPC * ET  # 2560 edges per core
ECH = 512  # edges per chunk
NCHK = EPC // ECH  # 5

B_X = 5  # exact blocks (of 128 ch) per head
NBLK = H * B_X  # 80 exact blocks
NPAIR_X = NBLK // 2  # 40 exact channel-block pairs
KQ = 1280  # kept quad eigencolumns (global top-|lambda|); tail mean-baked
NQ = KQ // 256  # 5 quad tiles (256 eigencolumns each)
CX = B_X * 128  # exact channels per head (640)

NCK = HC // 128  # 128 channel chunks for agg/z
FP8_S = 256.0  # W2 pre-scale so relu(Y2) fits fp8 e4m3
LSCALE = 0.8 / FP8_S  # logit scale applied in the exp
SQ_SCALE = 1.0 / LSCALE  # 320: folded into wq so sq8 weights are +-1
QA = 0.3989422804014327  # |t| ~ QA + QB t^2 fit for t ~ N(0,1)
QB = 0.3989422804014327

# Per-chunk op schedule: 40 exact pairs + 5 quad tiles, with engine
# assignment (Act abs / DVE relu for pairs; squares always Act — DVE
# cannot self-mult per the BIR verifier).  Act gets 20 pairs + 5
# squares, DVE 20 pairs, interleaved so neither engine sees long
# same-engine runs.  Entries: ('x', j, 'A'|'D') / ('q', qi, 'A').
N_PA = 20  # Act pairs
def _build_schedule():
    sched = []
    xj = 0
    qi = 0
    # 45 slots; quad tiles (Act) every 8 slots from 7 (late enough that
    # the wq DMA has landed before chunk-0 needs it)
    quad_slots = {7 + 8 * i for i in range(NQ)}
    na = nd = 0
    n_pd = NPAIR_X - N_PA
    for slot in range(NPAIR_X + NQ):
        if (slot in quad_slots and qi < NQ) or xj >= NPAIR_X:
            sched.append(('q', qi, 'A'))
            qi += 1
            continue
        pick = 'A' if na * n_pd <= nd * N_PA else 'D'
        if pick == 'A' and na >= N_PA:
            pick = 'D'
        if pick == 'D' and nd >= n_pd:
            pick = 'A'
        if pick == 'A':
            na += 1
        else:
            nd += 1
        sched.append(('x', xj, pick))
        xj += 1
    return sched


SCHED = _build_schedule()
# engine per exact pair (for host-side w/coef baking)
PAIR_ENG = {}
for kind, idx, eng in SCHED:
    if kind == 'x':
        PAIR_ENG[idx] = eng

_CACHE = {}


def _build_nc():
    import concourse.bacc as bacc
    import concourse.bass as bass
    import concourse.mybir as mybir
    import concourse.tile as tile

    f32 = mybir.dt.float32
    bf16 = mybir.dt.bfloat16
    fp8 = mybir.dt.float8e4
    AF = mybir.ActivationFunctionType
    OP = mybir.AluOpType
    PM = mybir.MatmulPerfMode

    nc = bacc.Bacc("TRN2", target_bir_lowering=False)

    W2W = NPAIR_X * 256  # 10240 exact-channel columns
    d_xT = nc.dram_tensor("xT", [128, EPC], bf16, kind="ExternalInput")
    d_w2 = nc.dram_tensor("w2", [128, W2W], bf16, kind="ExternalInput")
    d_wq = nc.dram_tensor("wq", [128, NQ, 2, 128], bf16, kind="ExternalInput")
    d_s8 = nc.dram_tensor("s8", [128, NPAIR_X, 2, H], fp8, kind="ExternalInput")
    d_sq8 = nc.dram_tensor("sq8", [128, NQ, 2, H], fp8, kind="ExternalInput")
    d_baseT = nc.dram_tensor("baseT", [16, EPC], f32, kind="ExternalInput")
    d_ident = nc.dram_tensor("ident", [16, 16], f32, kind="ExternalInput")
    d_s01t = nc.dram_tensor("s01t", [128, NSUB, NT], bf16, kind="ExternalInput")
    d_s01n = nc.dram_tensor("s01n", [128, NSUB, 128], bf16, kind="ExternalInput")
    d_xsrc = nc.dram_tensor("xsrc", [128, NSUB, F_IN], bf16, kind="ExternalInput")
    d_wagg = nc.dram_tensor("wagg", [128, 64, 128], bf16, kind="ExternalInput")
    d_wout = nc.dram_tensor("wout", [128, NCK, N_CLASS], bf16, kind="ExternalInput")
    d_biasc = nc.dram_tensor("biasc", [128, NCK], f32, kind="ExternalInput")
    d_z = nc.dram_tensor("z", [128, TPC, N_CLASS], f32, kind="ExternalOutput")

    with tile.TileContext(nc) as tc:
        with (
            tc.tile_pool(name="const", bufs=1) as cpool,
            tc.tile_pool(name="r8", bufs=8) as r8pool,
            tc.tile_pool(name="lbuf", bufs=3) as lpool,
            tc.tile_pool(name="xw", bufs=40) as xwpool,
            tc.tile_pool(name="rt", bufs=10) as rtpool,
            tc.tile_pool(name="rec", bufs=2) as recpool,
            tc.tile_pool(name="aux", bufs=1, space=bass.MemorySpace.PSUM) as auxp,
            tc.tile_pool(name="psT", bufs=1, space=bass.MemorySpace.PSUM) as psTp,
        ):
            xT = cpool.tile([128, EPC], bf16)
            w2 = cpool.tile([128, W2W], bf16)
            wq = cpool.tile([128, NQ, 2, 128], bf16)
            s8 = cpool.tile([128, NPAIR_X, 2, H], fp8)
            sq8 = cpool.tile([128, NQ, 2, H], fp8)
            baseT = cpool.tile([16, EPC], f32)
            ident = cpool.tile([16, 16], f32)
            s01t = cpool.tile([128, NSUB, NT], bf16)
            s01n = cpool.tile([128, NSUB, 128], bf16)
            xsrc = cpool.tile([128, NSUB, F_IN], bf16)
            wagg = cpool.tile([128, 64, 128], bf16)
            wout = cpool.tile([128, NCK, N_CLASS], bf16)
            biasc = cpool.tile([128, NCK], f32)
            # first-needed inputs first; w2 split so chunk-0 mains start
            # before the whole array arrives.  ident goes first (tiny) so
            # the act-table warmup below runs before the big DMAs land.
            nc.sync.dma_start(ident[:], d_ident[:])
            nc.sync.dma_start(xT[:, 0:ECH], d_xT[:, 0:ECH])
            nc.sync.dma_start(w2[:, 0:512], d_w2[:, 0:512])
            nc.sync.dma_start(s8[:], d_s8[:])
            nc.sync.dma_start(sq8[:], d_sq8[:])
            nc.sync.dma_start(wq[:], d_wq[:])
            nc.sync.dma_start(w2[:, 512:2048], d_w2[:, 512:2048])
            for kq in range(1, 5):
                nc.sync.dma_start(
                    w2[:, kq * 2048 : (kq + 1) * 2048],
                    d_w2[:, kq * 2048 : (kq + 1) * 2048],
                )
            nc.sync.dma_start(xT[:, ECH:], d_xT[:, ECH:])
            for sb, dr in [
                (baseT, d_baseT), (s01t, d_s01t),
                (s01n, d_s01n), (xsrc, d_xsrc), (wagg, d_wagg),
                (wout, d_wout), (biasc, d_biasc),
            ]:
                nc.sync.dma_start(sb[:], dr[:])

            # act-table warmup: a dummy Abs on ident pulls the (single)
            # LoadActFuncSet off the chunk-0 critical path; dummy matmuls
            # start the PE p-state ramp clock while the first DMAs land
            warm = cpool.tile([16, 16], f32)
            nc.scalar.activation(warm[:], ident[:], AF.Abs)

            P_sb = cpool.tile([128, NSUB, H], bf16)
            alpha_sb = cpool.tile([128, TPC, SUBT * H], f32)
            ssb = cpool.tile([128, TPC, H], bf16)
            G_sb = cpool.tile([128, TPC, 8, NT], bf16)
            z_sb = cpool.tile([128, TPC, N_CLASS], f32)

            def tile_tail(t):
                """den, reciprocal, alpha, xw, G, G-copy for node-tile t.

                For the last tile (t==3) the logit phase is over, so Act
                and DVE are idle: split the xw gathers three ways to get
                G_sb ready (and the agg phase started) as fast as possible.
                """
                last = t == TPC - 1
                with tc.high_priority():
                    sst_t = auxp.tile([128, 512], f32, tag="aux")
                    sst = sst_t[:, 0:H]
                    for s2 in range(SUBT):
                        nc.tensor.matmul(
                            sst,
                            s01t[:, t * SUBT + s2, :],
                            P_sb[:, t * SUBT + s2, :],
                            start=(s2 == 0),
                            stop=(s2 == SUBT - 1),
                        )
                    nc.vector.tensor_copy(ssb[:, t, :], sst)
                    # den: 5 single-shot matmuls, one-start in one aux alloc
                    dsp_t = auxp.tile([128, 512], f32, tag="aux")
                    dsp = dsp_t[:, 0 : SUBT * H].rearrange("p (a b) -> p a b", a=SUBT)
                    for s2 in range(SUBT):
                        nc.tensor.matmul(
                            dsp[:, s2, :],
                            s01n[:, t * SUBT + s2, :],
                            ssb[:, t, :],
                            start=(s2 == 0), stop=(s2 == SUBT - 1),
                        )
                    rec = recpool.tile([128, SUBT * H], f32, tag="rec")
                    nc.vector.reciprocal(rec[:], dsp_t[:, 0 : SUBT * H])
                    # alpha on Pool (SBUF-only engine work)
                    nc.gpsimd.tensor_tensor(
                        out=alpha_sb[:, t, :],
                        in0=P_sb[:, t * SUBT : (t + 1) * SUBT, :],
                        in1=rec[:],
                        op=OP.mult,
                    )
                # all alpha-scaled source gathers on Pool first, then the
                # G matmuls, so the span-ring slot is held briefly
                xws = {}
                nxw = [0]
                for pair in range(8):
                    for s2 in range(SUBT):
                        xw = xwpool.tile([128, 2, F_IN], bf16, tag="xw")
                        xws[pair, s2] = xw
                        for q2 in range(2):
                            h = 2 * pair + q2
                            al = alpha_sb[:, t, s2 * H + h : s2 * H + h + 1]
                            nc.gpsimd.tensor_scalar_mul(
                                xw[:, q2, :], xsrc[:, t * SUBT + s2, :], al
                            )
                # G in two single-bank halves (pairs 0-3, 4-7); one
                # start/stop per half, copied out as soon as each is done
                for gh in range(2):
                    G_t = auxp.tile([128, 512], f32, tag="aux")
                    G = G_t[:].rearrange("p (a b) -> p a b", a=4)
                    for ph in range(4):
                        pair = 4 * gh + ph
                        for s2 in range(SUBT):
                            nc.tensor.matmul(
                                G[:, ph, :],
                                xws[pair, s2][:].rearrange("p a b -> p (a b)"),
                                s01t[:, t * SUBT + s2, :],
                                start=(ph == 0 and s2 == 0),
                                stop=(ph == 3 and s2 == SUBT - 1),
                            )
                    if (2 * t + gh) % 2 == 0:
                        nc.scalar.copy(G_sb[:, t, 4 * gh : 4 * gh + 4, :], G[:])
                    else:
                        nc.vector.tensor_copy(G_sb[:, t, 4 * gh : 4 * gh + 4, :], G[:])

            # ---------------- logit phase, per 512-edge chunk ----------------
            PIPE = 3  # psT reduce trails the span/elementwise by 3 slots
            logit_pool = tc.tile_pool(name="span", bufs=3, space=bass.MemorySpace.PSUM)
            spanp = logit_pool.__enter__()
            for ck in range(NCHK):
                psT_t = psTp.tile([16, ECH], f32, tag="psT")
                psT = psT_t[:]
                e0 = ck * ECH
                nsc = len(SCHED)
                r8s = {}
                for si in range(nsc + PIPE):
                    if si < nsc:
                        kind, idx, eng = SCHED[si]
                        span = spanp.tile([128, 2, ECH], f32, tag="span")
                        r8 = r8pool.tile([128, 2, ECH], fp8, tag="r8")
                        if kind == 'x':
                            for kt in range(2):
                                cb = 2 * idx + kt
                                nc.tensor.matmul(
                                    span[:, kt, :],
                                    w2[:, cb * 128 : (cb + 1) * 128],
                                    xT[:, e0 : e0 + ECH],
                                )
                            if eng == 'A':
                                nc.scalar.activation(
                                    r8[:].rearrange("p a b -> p (a b)"),
                                    span[:].rearrange("p a b -> p (a b)"),
                                    AF.Abs,
                                )
                            else:
                                nc.vector.tensor_scalar(
                                    out=r8[:].rearrange("p a b -> p (a b)"),
                                    in0=span[:].rearrange("p a b -> p (a b)"),
                                    scalar1=0.0, scalar2=None, op0=OP.max,
                                )
                            r8s[si] = (r8, s8[:, idx, :, :])
                        else:
                            for kt in range(2):
                                nc.tensor.matmul(
                                    span[:, kt, :],
                                    wq[:, idx, kt, :],
                                    xT[:, e0 : e0 + ECH],
                                )
                            nc.scalar.activation(
                                r8[:].rearrange("p a b -> p (a b)"),
                                span[:].rearrange("p a b -> p (a b)"),
                                AF.Square,
                            )
                            r8s[si] = (r8, sq8[:, idx, :, :])
                    if si >= PIPE:
                        r8p, red = r8s.pop(si - PIPE)
                        nc.tensor.matmul(
                            psT,
                            red,
                            r8p[:],
                            start=(si - PIPE == 0),
                            stop=(si - PIPE == nsc - 1),
                            perf_mode=PM.DoubleRow,
                        )
                # chunk drain: logits -> P, seg-sums; tile tail when complete
                with tc.high_priority():
                    Lb = lpool.tile([16, ECH], f32, tag="lbuf")
                    nc.vector.tensor_tensor(
                        out=Lb[:], in0=psT,
                        in1=baseT[:, e0 : e0 + ECH], op=OP.add,
                    )
                    ptrt_t = psTp.tile([128, 512], f32, tag="psT")
                    ptrt = ptrt_t[:, 0:64].rearrange("p (a b) -> p a b", a=4)
                    for i in range(4):
                        nc.tensor.matmul(
                            ptrt[:, i, :],
                            Lb[:, i * 128 : (i + 1) * 128], ident[:],
                            is_transpose=True,
                            start=(i == 0), stop=(i == 3),
                        )
                    nc.scalar.activation(
                        P_sb[:, 4 * ck : 4 * ck + 4, :],
                        ptrt[:],
                        AF.Exp,
                        scale=LSCALE,
                    )
                # tails run at normal priority so G matmuls don't preempt
                # the span pipeline on the PE
                for i in range(4):
                    s = 4 * ck + i
                    t, sid = divmod(s, SUBT)
                    if sid == SUBT - 1:
                        tile_tail(t)

            # ---------------- aggregation phase ----------------
            logit_pool.__exit__(None, None, None)
            nz = [0]
            zp_t = auxp.tile([128, 512], f32, tag="aux")
            zp = zp_t[:, 0 : TPC * N_CLASS]
            rts = {}
            ZLAG = 2
            with tc.tile_pool(
                name="aggsp", bufs=6, space=bass.MemorySpace.PSUM
            ) as aggsp:
                for kk in range(NCK + ZLAG):
                    if kk < NCK:
                        span = aggsp.tile([128, TPC, NT], f32, tag="aspan")
                        rt = rtpool.tile([128, TPC, NT], bf16, tag="rt")
                        rts[kk] = rt
                        q2 = (kk // 8) % 2
                        jcol = (kk // 16) * 8 + kk % 8
                        pair = (kk // 8) // 2
                        nc.tensor.matmul(
                            span[:],
                            wagg[q2 * 64 : (q2 + 1) * 64, jcol, :],
                            G_sb[q2 * 64 : (q2 + 1) * 64, :, pair, :],
                        )
                        if (kk % 15) % 2 == 0:
                            nc.scalar.activation(
                                rt[:].rearrange("p a b -> p (a b)"),
                                span[:].rearrange("p a b -> p (a b)"),
                                AF.Relu,
                                bias=biasc[:, kk : kk + 1],
                            )
                        else:
                            nc.vector.tensor_scalar(
                                out=rt[:].rearrange("p a b -> p (a b)"),
                                in0=span[:].rearrange("p a b -> p (a b)"),
                                scalar1=biasc[:, kk : kk + 1],
                                scalar2=0.0, op0=OP.add, op1=OP.max,
                            )
                    # z matmuls trail so PE never waits on the relus
                    if kk >= ZLAG:
                        kz = kk - ZLAG
                        rtp = rts.pop(kz)
                        for t in range(TPC):
                            nz[0] += 1
                            nc.tensor.matmul(
                                zp[:, t * N_CLASS : (t + 1) * N_CLASS],
                                rtp[:, t, :],
                                wout[:, kz, :],
                                start=(nz[0] == 1),
                                stop=(nz[0] == NCK * TPC),
                            )
            nc.vector.tensor_copy(
                z_sb[:].rearrange("p a b -> p (a b)"),
                zp[:, 0 : TPC * N_CLASS],
            )
            nc.sync.dma_start(d_z[:], z_sb[:])

    nc.compile()
    return nc


def _balance_tiles(deg):
    """Assign nodes to 32 tiles: 128 nodes and exactly 640 edges each."""
    T = N_CORES * TPC
    order = np.argsort(-deg, kind="stable")
    tiles = [[] for _ in range(T)]
    ebud = np.full(T, ET)
    nbud = np.full(T, NT)
    for n in order:
        cand = np.where(nbud > 0)[0]
        best = cand[np.argmax(ebud[cand] - nbud[cand])]
        tiles[best].append(int(n))
        ebud[best] -= deg[n]
        nbud[best] -= 1
    if not np.all(ebud == 0):
        rng = np.random.default_rng(0)
        for _ in range(500000):
            if np.all(ebud == 0):
                break
            hi = int(np.argmin(ebud))
            lo = int(np.argmax(ebud))
            a, b = tiles[hi], tiles[lo]
            ia = int(rng.integers(len(a)))
            ib = int(rng.integers(len(b)))
            delta = deg[a[ia]] - deg[b[ib]]
            if 0 < delta <= ebud[lo] - ebud[hi]:
                a[ia], b[ib] = b[ib], a[ia]
                ebud[hi] += delta
                ebud[lo] -= delta
    assert np.all(ebud == 0), f"tile balancing failed: {ebud}"
    return tiles


def _prep_inputs(x, edge_index, W_l, W_r, att, bias_gat, W_out, b_out):
    import ml_dtypes

    bf16 = ml_dtypes.bfloat16
    fp8 = ml_dtypes.float8_e4m3
    x = np.asarray(x, np.float32)
    W_l = np.asarray(W_l, np.float32)
    W_r = np.asarray(W_r, np.float32)
    att = np.asarray(att, np.float32)
    bias_gat = np.asarray(bias_gat, np.float32)
    W_out = np.asarray(W_out, np.float32)

    src = np.concatenate([np.asarray(edge_index[0]), np.arange(N_NODES)]).astype(
        np.int64
    )
    dst = np.concatenate([np.asarray(edge_index[1]), np.arange(N_NODES)]).astype(
        np.int64
    )
    deg = np.bincount(dst, minlength=N_NODES)
    tiles = _balance_tiles(deg)

    attf = att.reshape(HC).astype(np.float64)
    Wcat = np.vstack([W_l, W_r]).astype(np.float64)  # [128, HC]
    sig_edge = np.linalg.norm(Wcat, axis=0)  # per-channel sigma, regular edge
    Wsum = (W_l + W_r).astype(np.float64)
    sig_self = np.linalg.norm(Wsum, axis=0)  # self-loop sigma

    # ---- split channels per head: exact top CX by |a|*sigma, quad rest
    score = (np.abs(attf) * sig_edge).reshape(H, C)
    exact_idx = np.empty((H, CX), np.int64)  # global channel ids
    quad_sets = []
    for h in range(H):
        o = np.argsort(-score[h], kind="stable")
        exact_idx[h] = h * C + o[:CX]
        quad_sets.append(h * C + o[CX:])

    # exact blocks: head-major, 5 blocks per head
    blocks = np.empty((NBLK, 128), np.int64)
    blk2head = np.empty(NBLK, np.int64)
    for h in range(H):
        for j in range(B_X):
            g = h * B_X + j
            blocks[g] = exact_idx[h, j * 128 : (j + 1) * 128]
            blk2head[g] = h

    # W2: exact-channel columns, |a| and FP8_S folded
    W2 = np.empty((128, NBLK * 128), np.float64)
    for g in range(NBLK):
        ch = blocks[g]
        W2[:, g * 128 : (g + 1) * 128] = (
            Wcat[:, ch] * np.abs(attf[ch])[None, :] * FP8_S
        )
    W2 = W2.astype(bf16)

    # s8 reduce matrix: sign(a_c), scaled 0.5 for Act (abs-stored) pairs
    s8 = np.zeros((128, NPAIR_X, 2, H), np.float32)
    for j in range(NPAIR_X):
        w = 0.5 if PAIR_ENG[j] == 'A' else 1.0
        for kt in range(2):
            g = 2 * j + kt
            h = blk2head[g]
            s8[:, j, kt, h] = w * np.sign(attf[blocks[g]])
    s8 = s8.astype(fp8)

    # ---- quad path: M_h eigendecomp, keep global top-KQ |lambda| columns
    # (sqrt(320|lam|) folded into wq; sq8 weights exactly +-1), mean-bake
    # the dropped tail per edge-type into qconst.
    qconst_edge = np.zeros(H, np.float64)
    qconst_self = np.zeros(H, np.float64)
    eigs = []  # (|lam|, sign, h, u[128])
    for h in range(H):
        qc = quad_sets[h]
        aq = attf[qc]
        sq = sig_edge[qc]
        M = (0.4 * QB) * ((Wcat[:, qc] * (aq / sq)[None, :]) @ Wcat[:, qc].T)
        lam, U = np.linalg.eigh(M)
        for k in range(128):
            eigs.append((abs(lam[k]), np.sign(lam[k]), h, U[:, k]))
        qconst_edge[h] = 0.4 * QA * np.sum(aq * sq)
        qconst_self[h] = 0.4 * QA * np.sum(aq * sig_self[qc])
    eigs.sort(key=lambda e: -e[0])
    wq = np.zeros((128, NQ, 2, 128), np.float64)
    sq8 = np.zeros((128, NQ, 2, H), np.float32)
    for i, (al, sgn, h, u) in enumerate(eigs[:KQ]):
        ti, rest = divmod(i, 256)
        kt, p = divmod(rest, 128)
        wq[:, ti, kt, p] = u * np.sqrt(SQ_SCALE * al)
        sq8[p, ti, kt, h] = sgn
    for al, sgn, h, u in eigs[KQ:]:
        # E[(u^T f)^2] = 1 for independent src/dst; ||u_a+u_b||^2 for
        # self-loops (f = [x; x])
        qconst_edge[h] += sgn * al
        qconst_self[h] += sgn * al * float(np.sum((u[:64] + u[64:]) ** 2))
    wq = wq.astype(bf16)
    sq8 = sq8.astype(fp8)

    # ---- linear-part bake: coef per channel
    # 0.6 for quad channels and Act(abs)-pair channels, 0.2 for DVE(relu)
    coef = np.full(HC, 0.6, np.float64)
    for j in range(NPAIR_X):
        if PAIR_ENG[j] != 'A':
            for kt in range(2):
                coef[blocks[2 * j + kt]] = 0.2
    attc = (attf * coef).reshape(H, C)
    ul = np.einsum("fhc,hc->fh", W_l.reshape(F_IN, H, C).astype(np.float64), attc)
    ur = np.einsum("fhc,hc->fh", W_r.reshape(F_IN, H, C).astype(np.float64), attc)
    sl = x.astype(np.float64) @ ul
    sr = x.astype(np.float64) @ ur

    # wagg: chunk kk of head h=kk//8 stored at partitions (h%2)*64..+64,
    # column j=(kk//16)*8+kk%8
    wagg = np.zeros((128, 64, 128), np.float32)
    for kk in range(NCK):
        q2 = (kk // 8) % 2
        jcol = (kk // 16) * 8 + kk % 8
        wagg[q2 * 64 : (q2 + 1) * 64, jcol, :] = W_l[:, kk * 128 : (kk + 1) * 128]
    wagg = wagg.astype(bf16)

    wout = np.ascontiguousarray(
        W_out.reshape(NCK, 128, N_CLASS).transpose(1, 0, 2)
    ).astype(bf16)
    biasc = np.ascontiguousarray(bias_gat.reshape(NCK, 128).T).astype(np.float32)
    ident = np.eye(16, dtype=np.float32)

    in_maps = []
    node_map = np.empty((N_CORES, TPC, NT), np.int64)
    for core in range(N_CORES):
        xT = np.zeros((128, EPC), np.float32)
        baseT = np.zeros((16, EPC), np.float32)
        s01t = np.zeros((128, NSUB, NT), np.float32)
        s01n = np.zeros((128, NSUB, 128), np.float32)
        xsrc_a = np.zeros((128, NSUB, F_IN), np.float32)
        for tl in range(TPC):
            tg = core * TPC + tl
            nodes = np.asarray(tiles[tg], np.int64)
            node_map[core, tl] = nodes
            # local dst index per node
            loc = np.full(N_NODES, -1, np.int64)
            loc[nodes] = np.arange(NT)
            mask = loc[dst] >= 0
            mask &= np.isin(dst, nodes)
            es = src[mask]
            ed = loc[dst[mask]]
            o = np.argsort(ed, kind="stable")
            es, ed = es[o], ed[o]
            assert len(es) == ET, f"tile {tg}: {len(es)} edges"
            is_self = es == nodes[ed]
            qc = np.where(is_self[None, :], qconst_self[:, None],
                          qconst_edge[:, None])  # [H, ET]
            slot = np.arange(ET)
            ssub, p = slot // 128 + tl * SUBT, slot % 128
            e0 = tl * ET
            xT[0:64, e0 + slot] = x[es].T
            xT[64:128, e0 + slot] = x[nodes[ed]].T
            baseT[:, e0 + slot] = SQ_SCALE * (
                (sl[es] + sr[nodes[ed]]).T + qc
            )
            s01t[p, ssub, ed] = 1.0
            s01n[ed, ssub, p] = 1.0
            xsrc_a[p, ssub, :] = x[es]
        in_maps.append(
            {
                "xT": xT.astype(bf16),
                "w2": W2,
                "wq": wq,
                "s8": s8,
                "sq8": sq8,
                "baseT": baseT,
                "ident": ident,
                "s01t": s01t.astype(bf16),
                "s01n": s01n.astype(bf16),
                "xsrc": xsrc_a.astype(bf16),
                "wagg": wagg,
                "wout": wout,
                "biasc": biasc,
            }
        )
    return in_maps, node_map


def kernel(**inputs):
    from concourse.bass_utils import run_bass_kernel_spmd

    if "nc" not in _CACHE:
        _CACHE["nc"] = _build_nc()
    nc = _CACHE["nc"]

    in_maps, node_map = _prep_inputs(**inputs)
    res = run_bass_kernel_spmd(nc, in_maps, list(range(N_CORES)))
    b_out = np.asarray(inputs["b_out"], np.float32)
    z = np.empty((N_NODES, N_CLASS), np.float32)
    for core in range(N_CORES):
        zc = np.asarray(res.results[core]["z"], np.float32)  # [128, TPC, 20]
        for tl in range(TPC):
            z[node_map[core, tl]] = zc[:, tl, :]
    return z + b_out
